# revision 27
# baseline (speedup 1.0000x reference)
"""Trainium2 Bass kernel for nn_ContrastiveLoss (SimCLR-style NT-Xent loss).

Math: z = concat(f1, f2) [2B, D]; zn = z / ||z||_row;
logits = zn @ zn.T / T; labels[i] = i mod B;
loss = mean_i(logsumexp(logits[i, :]) - logits[i, label_i]).

Distribution: data-parallel over rows of z across 8 NeuronCores. Each
core computes its 1024-row block of logits against all 8192 columns
(bf16 GEMM on the PE), with the softmax statistics fused on the fly:
exp(2*cos) with per-instruction free-dim accumulation on the Scalar
engine, so the full 8192x8192 logits matrix is never materialized.
The target logit is computed separately as an elementwise row-dot
(t_r = 2 * zn_r . zn_label(r)), so no gather is needed. Row norms are
computed on-device from the transposed operand via Square (ACT) +
ones-matmul partition reduction (PE). The host only does layout
(concat/slice/permute/transpose), sharding, and the final 8-way sum.
"""

import numpy as np

import concourse.bass as bass
import concourse.mybir as mybir
import concourse.tile as tile
from concourse.bass_utils import run_bass_kernel_spmd
from concourse.masks import make_identity
from concourse.vector_clock import ScopedClock

F32 = mybir.dt.float32
BF16 = mybir.dt.bfloat16
AF = mybir.ActivationFunctionType
ALU = mybir.AluOpType

B = 4096
D = 512
N2 = 2 * B          # 8192 rows of z
NCORES = 8
ROWS = N2 // NCORES  # 1024 rows per core
MT = ROWS // 128     # 8 m-tiles per core
KT = D // 128        # 4 k-tiles
CHUNK = 1024         # GEMM column chunk (2 PSUM banks)
NCH = N2 // CHUNK    # 8 GEMM column chunks
SCHUNK = 1024        # normalize/scale column chunk
NSC = N2 // SCHUNK   # 8 scale chunks
TEMP_INV = 2.0       # 1 / temperature


# ---------------------------------------------------------------------------
# Patches for this toolchain build:
# 1) walrus CoreV2/V3 codegen only accepts ONE sync wait per instruction;
#    Tile attaches several (tail drain, multi-dep DMAs). Split extras onto
#    standalone EventSemaphore instructions placed immediately before the
#    overloaded instruction (same engine, same basic block) — blocking at
#    engine-issue time is strictly more conservative and deadlock-free
#    because Tile's per-engine streams preserve global dependency order.
# ---------------------------------------------------------------------------
_MAX_WAITS = 1
_patched = False


def _patched_drain_and_barrier(self, tick_clock, wait_clock):
    nc = self.nc
    drain_inst = nc.sync.drain()
    wait_clock.add_sem_waits(
        drain_inst.ins, ScopedClock({None: tick_clock.global_clock})
    )
    si = drain_inst.ins.sync_info
    if si is not None and si.on_wait and len(si.on_wait) > _MAX_WAITS:
        waits = list(si.on_wait)
        si.on_wait = waits[:_MAX_WAITS]
        for i in range(_MAX_WAITS, len(waits), _MAX_WAITS):
            extra = nc.sync.drain()
            extra.ins.sync_info = mybir.SyncInfo(
                on_wait=waits[i : i + _MAX_WAITS], on_update=[]
            )
    nc.all_engine_barrier()
    assert self.sems is not None
    popped = nc._tile_sem_poison_stack.pop()
    assert popped is self._sem_poison
    nc.clear_and_free_semaphores(list(self.sems.allocated().values()))
    nc.all_engine_barrier()


def _apply_patches():
    global _patched
    if _patched:
        return
    tile.TileContext._drain_and_barrier = _patched_drain_and_barrier
    _patched = True


def _split_waits(nc):
    n = 0
    for fn in nc.m.functions:
        for bb in fn.blocks:
            insts = bb.instructions
            if not any(
                i.sync_info
                and i.sync_info.on_wait
                and len(i.sync_info.on_wait) > _MAX_WAITS
                for i in insts
            ):
                continue
            out = []
            for inst in insts:
                si = inst.sync_info
                if si and si.on_wait and len(si.on_wait) > _MAX_WAITS:
                    waits = list(si.on_wait)
                    for w in waits[:-_MAX_WAITS]:
                        n += 1
                        ev = mybir.InstEventSemaphore(
                            name=f"WSPLIT-{n}", ins=[], outs=[]
                        )
                        ev.engine = inst.engine
                        ev.sync_info = mybir.SyncInfo(on_wait=[w], on_update=[])
                        out.append(ev)
                    si.on_wait = waits[-_MAX_WAITS:]
                out.append(inst)
            bb.instructions = out
    return n


# ---------------------------------------------------------------------------
# Device kernel (identical program on all 8 cores; per-core data differs)
# ---------------------------------------------------------------------------
def _build_nc():
    _apply_patches()
    nc = bass.Bass()

    # zt:   [D, N2] f32 — z rows transposed; per-core column order:
    #       own rows first, then (cores 4-7) label rows, then the rest
    # tsel: [128, 2] f32 — one-hot: target diagonal at col 0 or col 1024
    zt = nc.declare_dram_parameter("zt", [D, N2], F32, isOutput=False)
    tsel = nc.declare_dram_parameter("tsel", [128, 2], F32, isOutput=False)
    out = nc.declare_dram_parameter("out", [128, MT], F32, isOutput=True)


    with tile.TileContext(nc) as tc:
        with (
            tc.tile_pool(name="persist", bufs=1) as persist,
            tc.tile_pool(name="ztst", bufs=4) as ztst_pool,
            tc.tile_pool(name="sq", bufs=2) as sq_pool,
            tc.tile_pool(name="invb", bufs=2) as invb_pool,
            tc.tile_pool(name="small", bufs=2) as small_pool,
            tc.tile_pool(name="psum", bufs=4, space="PSUM") as psum_pool,
        ):
            # persistent tensors
            znT = [
                persist.tile([128, N2], BF16, tag=f"znT{k}", name=f"znT{k}") for k in range(KT)
            ]
            ones = persist.tile([128, 128], BF16, tag="ones")
            nc.vector.memset(ones, 1.0)
            acc = persist.tile([128, MT, NCH], F32, tag="acc")
            d0 = persist.tile([128, MT], F32, tag="d0")
            d1 = persist.tile([128, MT], F32, tag="d1")
            ident = persist.tile([128, 128], BF16, tag="ident")
            make_identity(nc, ident)
            tselt = persist.tile([128, 2], F32, tag="tselt")
            nc.sync.dma_start(out=tselt, in_=tsel.ap())
            escr = persist.tile([128, 2, MT, 128], BF16, tag="escr")

            # ---- per column-chunk: cast-DMA zt to bf16, norms^2 via
            #      square (DVE) + ones-matmul (PE), inv = exp(-0.5*ln(ss))
            #      on ACT (replicated across partitions), scale to znT --
            # Software-pipelined by EMISSION order: chunk cc's scales are
            # emitted after chunk cc+1's squares, so in the scheduler's
            # priority order a later chunk's squares never head-of-line
            # block an earlier chunk's scales on the DVE.
            def emit_scales(pend):
                pcs, pztst, pinvb = pend
                for kt in range(KT):
                    nc.vector.tensor_mul(znT[kt][:, pcs], pztst[kt], pinvb)

            # Graduated chunk sizes: small first chunks shorten the
            # first-scale latency so the GEMM starts earlier.
            sizes = [512, 512] + [SCHUNK] * ((N2 - 1024) // SCHUNK)
            pending = None
            off = 0
            for cc, size in enumerate(sizes):
                cs = slice(off, off + size)
                off += size
                ps = psum_pool.tile([128, CHUNK], F32, name="ps")
                ztst = {}
                for kt in range(KT):
                    st = ztst_pool.tile([128, size], BF16, tag=f"zt{kt}", name=f"zt{kt}")
                    nc.gpsimd.dma_start(
                        out=st, in_=zt.ap()[kt * 128 : (kt + 1) * 128, cs]
                    )
                    ztst[kt] = st
                    sq = sq_pool.tile([128, size], BF16, tag="sq", name="sq")
                    nc.vector.tensor_mul(sq, st, st)
                    for n in range(size // 512):
                        nc.tensor.matmul(
                            ps[:, n * 512 : (n + 1) * 512],
                            ones,
                            sq[:, n * 512 : (n + 1) * 512],
                            start=(kt == 0),
                            stop=(kt == KT - 1),
                        )
                # inv-norm, replicated across partitions by the ones-matmul:
                # inv = exp(-0.5 * ln(ss)) (one ACT table set, full width)
                lnb = small_pool.tile([128, size], F32, tag="lnb", name="lnb")
                nc.scalar.activation(out=lnb, in_=ps[:, 0:size], func=AF.Ln)
                invb = invb_pool.tile([128, size], BF16, tag="invb", name="invb")
                nc.scalar.activation(out=invb, in_=lnb, func=AF.Exp, scale=-0.5)
                if pending is not None:
                    emit_scales(pending)
                pending = (cs, ztst, invb)
            emit_scales(pending)

            # ---- GEMM + fused exp/accumulate --------------------------------
            # logits chunk = znT_own(m).T @ znT_all(chunk); exp(2x) with
            # free-dim accumulation, written back in-place to PSUM.
            for nb in range(NCH):
                for m in range(MT):
                    ps = psum_pool.tile([128, CHUNK], F32)
                    for kt in range(KT):
                        lhsT = znT[kt][:, m * 128 : (m + 1) * 128]
                        for n in range(CHUNK // 512):
                            col = nb * CHUNK + n * 512
                            nc.tensor.matmul(
                                ps[:, n * 512 : (n + 1) * 512],
                                lhsT,
                                znT[kt][:, col : col + 512],
                                start=(kt == 0),
                                stop=(kt == KT - 1),
                            )
                    if nb < 2:
                        # stash exp(2*logit) of the target-diagonal window in
                        # SBUF before the in-place exp below overwrites PSUM;
                        # the diagonal is extracted after the scale stream.
                        nc.scalar.activation(
                            out=escr[:, nb, m, :],
                            in_=ps[:, m * 128 : (m + 1) * 128],
                            func=AF.Exp, scale=TEMP_INV,
                        )
                    nc.scalar.activation(
                        out=ps, in_=ps, func=AF.Exp, scale=TEMP_INV,
                        accum_out=acc[:, m, nb : nb + 1],
                    )
                if nb == 2:
                    # target-diagonal extraction (inputs ready after nb=1;
                    # emitted here so it overlaps the remaining GEMM instead
                    # of running in the kernel tail)
                    for w in range(2):
                        dtarget = d0 if w == 0 else d1
                        for m in range(MT):
                            dsc = sq_pool.tile([128, 128], F32, tag="dsc")
                            nc.vector.tensor_mul(dsc, escr[:, w, m, :], ident)
                            nc.vector.tensor_reduce(
                                out=dtarget[:, m : m + 1], in_=dsc,
                                axis=mybir.AxisListType.X, op=ALU.add,
                            )

            # d0/d1 hold exp(2*cos); recover the logit via ln
            nc.scalar.activation(out=d0, in_=d0, func=AF.Ln)
            nc.scalar.activation(out=d1, in_=d1, func=AF.Ln)
            t2a = persist.tile([128, MT], F32, tag="t2a")
            nc.vector.tensor_scalar_mul(t2a, d0, tselt[:, 0:1])
            t2b = persist.tile([128, MT], F32, tag="t2b")
            nc.vector.tensor_scalar_mul(t2b, d1, tselt[:, 1:2])
            t2 = persist.tile([128, MT], F32, tag="t2")
            nc.vector.tensor_add(t2, t2a, t2b)

            # ---- finalize: lse = ln(sum exp), partials = lse - t ---------
            ssum = persist.tile([128, MT], F32, tag="ssum")
            nc.vector.tensor_reduce(
                out=ssum, in_=acc, axis=mybir.AxisListType.X, op=ALU.add
            )
            lse = persist.tile([128, MT], F32, tag="lse")
            nc.scalar.activation(out=lse, in_=ssum, func=AF.Ln)
            diff = persist.tile([128, MT], F32, tag="diff")
            nc.vector.tensor_sub(diff, lse, t2)
            nc.sync.dma_start(out=out.ap(), in_=diff)

    _split_waits(nc)
    return nc


_nc_cache = None


def _get_nc():
    global _nc_cache
    if _nc_cache is None:
        _nc_cache = _build_nc()
    return _nc_cache


# ---------------------------------------------------------------------------
# Host wrapper: shard, run SPMD on cores 0-7, reduce
# ---------------------------------------------------------------------------
def kernel(features_1, features_2, _trace=False):
    f1 = np.ascontiguousarray(np.asarray(features_1, dtype=np.float32))
    f2 = np.ascontiguousarray(np.asarray(features_2, dtype=np.float32))
    assert f1.shape == (B, D) and f2.shape == (B, D)
    z = np.concatenate([f1, f2], axis=0)  # [N2, D]

    in_maps = []
    allrows = np.arange(N2)
    for c in range(NCORES):
        own_lo = c * ROWS
        lab_lo = (c % (B // ROWS)) * ROWS
        own_idx = allrows[own_lo : own_lo + ROWS]
        if lab_lo == own_lo:
            rest = np.concatenate([allrows[:own_lo], allrows[own_lo + ROWS :]])
            R = np.concatenate([own_idx, rest])
            sel = (1.0, 0.0)
        else:
            lab_idx = allrows[lab_lo : lab_lo + ROWS]
            keep = np.ones(N2, dtype=bool)
            keep[own_idx] = False
            keep[lab_idx] = False
            R = np.concatenate([own_idx, lab_idx, allrows[keep]])
            sel = (0.0, 1.0)
        in_maps.append(
            {
                "zt": np.ascontiguousarray(z[R].T),
                "tsel": np.tile(np.array(sel, np.float32), (128, 1)),
            }
        )

    nc = _get_nc()
    res = run_bass_kernel_spmd(
        nc, in_maps, core_ids=list(range(NCORES)), trace=_trace
    )
    total = np.float64(0.0)
    for c in range(NCORES):
        total += res.results[c]["out"].astype(np.float64).sum()
    loss = np.float32(total / N2)
    if _trace:
        return loss, res
    return loss


# revision 28
# speedup vs baseline: 1.0275x; 1.0275x over previous
"""Trainium2 Bass kernel for nn_ContrastiveLoss (SimCLR-style NT-Xent loss).

Math: z = concat(f1, f2) [2B, D]; zn = z / ||z||_row;
logits = zn @ zn.T / T; labels[i] = i mod B;
loss = mean_i(logsumexp(logits[i, :]) - logits[i, label_i]).

Distribution: data-parallel over rows of z across 8 NeuronCores. Each
core computes its 1024-row block of logits against all 8192 columns
(bf16 GEMM on the PE), with the softmax statistics fused on the fly:
exp(2*cos) with per-instruction free-dim accumulation on the Scalar
engine, so the full 8192x8192 logits matrix is never materialized.
The target logit is computed separately as an elementwise row-dot
(t_r = 2 * zn_r . zn_label(r)), so no gather is needed. Row norms are
computed on-device from the transposed operand via Square (ACT) +
ones-matmul partition reduction (PE). The host only does layout
(concat/slice/permute/transpose), sharding, and the final 8-way sum.
"""

import numpy as np

import concourse.bass as bass
import concourse.mybir as mybir
import concourse.tile as tile
from concourse.bass_utils import run_bass_kernel_spmd
from concourse.masks import make_identity
from concourse.vector_clock import ScopedClock

F32 = mybir.dt.float32
BF16 = mybir.dt.bfloat16
AF = mybir.ActivationFunctionType
ALU = mybir.AluOpType

B = 4096
D = 512
N2 = 2 * B          # 8192 rows of z
NCORES = 8
ROWS = N2 // NCORES  # 1024 rows per core
MT = ROWS // 128     # 8 m-tiles per core
KT = D // 128        # 4 k-tiles
CHUNK = 1024         # GEMM column chunk (2 PSUM banks)
NCH = N2 // CHUNK    # 8 GEMM column chunks
SCHUNK = 1024        # normalize/scale column chunk
NSC = N2 // SCHUNK   # 8 scale chunks
TEMP_INV = 2.0       # 1 / temperature


# ---------------------------------------------------------------------------
# Patches for this toolchain build:
# 1) walrus CoreV2/V3 codegen only accepts ONE sync wait per instruction;
#    Tile attaches several (tail drain, multi-dep DMAs). Split extras onto
#    standalone EventSemaphore instructions placed immediately before the
#    overloaded instruction (same engine, same basic block) — blocking at
#    engine-issue time is strictly more conservative and deadlock-free
#    because Tile's per-engine streams preserve global dependency order.
# ---------------------------------------------------------------------------
_MAX_WAITS = 1
_patched = False


def _patched_drain_and_barrier(self, tick_clock, wait_clock):
    nc = self.nc
    drain_inst = nc.sync.drain()
    wait_clock.add_sem_waits(
        drain_inst.ins, ScopedClock({None: tick_clock.global_clock})
    )
    si = drain_inst.ins.sync_info
    if si is not None and si.on_wait and len(si.on_wait) > _MAX_WAITS:
        waits = list(si.on_wait)
        si.on_wait = waits[:_MAX_WAITS]
        for i in range(_MAX_WAITS, len(waits), _MAX_WAITS):
            extra = nc.sync.drain()
            extra.ins.sync_info = mybir.SyncInfo(
                on_wait=waits[i : i + _MAX_WAITS], on_update=[]
            )
    nc.all_engine_barrier()
    assert self.sems is not None
    popped = nc._tile_sem_poison_stack.pop()
    assert popped is self._sem_poison
    nc.clear_and_free_semaphores(list(self.sems.allocated().values()))
    nc.all_engine_barrier()


def _apply_patches():
    global _patched
    if _patched:
        return
    tile.TileContext._drain_and_barrier = _patched_drain_and_barrier
    _patched = True


def _split_waits(nc):
    n = 0
    for fn in nc.m.functions:
        for bb in fn.blocks:
            insts = bb.instructions
            if not any(
                i.sync_info
                and i.sync_info.on_wait
                and len(i.sync_info.on_wait) > _MAX_WAITS
                for i in insts
            ):
                continue
            out = []
            for inst in insts:
                si = inst.sync_info
                if si and si.on_wait and len(si.on_wait) > _MAX_WAITS:
                    waits = list(si.on_wait)
                    for w in waits[:-_MAX_WAITS]:
                        n += 1
                        ev = mybir.InstEventSemaphore(
                            name=f"WSPLIT-{n}", ins=[], outs=[]
                        )
                        ev.engine = inst.engine
                        ev.sync_info = mybir.SyncInfo(on_wait=[w], on_update=[])
                        out.append(ev)
                    si.on_wait = waits[-_MAX_WAITS:]
                out.append(inst)
            bb.instructions = out
    return n


# ---------------------------------------------------------------------------
# Device kernel (identical program on all 8 cores; per-core data differs)
# ---------------------------------------------------------------------------
def _build_nc():
    _apply_patches()
    nc = bass.Bass()

    # zt:   [D, N2] f32 — z rows transposed; per-core column order:
    #       own rows first, then (cores 4-7) label rows, then the rest
    # tsel: [128, 2] f32 — one-hot: target diagonal at col 0 or col 1024
    zt = nc.declare_dram_parameter("zt", [D, N2], F32, isOutput=False)
    tsel = nc.declare_dram_parameter("tsel", [128, 2], F32, isOutput=False)
    out = nc.declare_dram_parameter("out", [128, MT], F32, isOutput=True)


    with tile.TileContext(nc) as tc:
        with (
            tc.tile_pool(name="persist", bufs=1) as persist,
            tc.tile_pool(name="ztst", bufs=4) as ztst_pool,
            tc.tile_pool(name="sq", bufs=2) as sq_pool,
            tc.tile_pool(name="invb", bufs=2) as invb_pool,
            tc.tile_pool(name="small", bufs=2) as small_pool,
            tc.tile_pool(name="psum", bufs=4, space="PSUM") as psum_pool,
        ):
            # persistent tensors
            znT = [
                persist.tile([128, N2], BF16, tag=f"znT{k}", name=f"znT{k}") for k in range(KT)
            ]
            ones = persist.tile([128, 128], BF16, tag="ones")
            nc.vector.memset(ones, 1.0)
            acc = persist.tile([128, MT, NCH], F32, tag="acc")
            d0 = persist.tile([128, MT], F32, tag="d0")
            d1 = persist.tile([128, MT], F32, tag="d1")
            ident = persist.tile([128, 128], BF16, tag="ident")
            make_identity(nc, ident)
            tselt = persist.tile([128, 2], F32, tag="tselt")
            nc.sync.dma_start(out=tselt, in_=tsel.ap())
            escr = persist.tile([128, 2, MT, 128], BF16, tag="escr")

            # ---- per column-chunk: cast-DMA zt to bf16, norms^2 via
            #      square (DVE) + ones-matmul (PE), inv = exp(-0.5*ln(ss))
            #      on ACT (replicated across partitions), scale to znT --
            # Software-pipelined by EMISSION order: chunk cc's scales are
            # emitted after chunk cc+1's squares, so in the scheduler's
            # priority order a later chunk's squares never head-of-line
            # block an earlier chunk's scales on the DVE.
            def emit_scales(pend):
                pcs, pztst, pinvb = pend
                for kt in range(KT):
                    nc.vector.tensor_mul(znT[kt][:, pcs], pztst[kt], pinvb)

            sizes = [SCHUNK] * NSC
            pending = None
            off = 0
            for cc, size in enumerate(sizes):
                cs = slice(off, off + size)
                off += size
                ps = psum_pool.tile([128, CHUNK], F32, name="ps")
                ztst = {}
                for kt in range(KT):
                    st = ztst_pool.tile([128, size], BF16, tag=f"zt{kt}", name=f"zt{kt}")
                    nc.gpsimd.dma_start(
                        out=st, in_=zt.ap()[kt * 128 : (kt + 1) * 128, cs]
                    )
                    ztst[kt] = st
                    sq = sq_pool.tile([128, size], BF16, tag="sq", name="sq")
                    nc.vector.tensor_mul(sq, st, st)
                    for n in range(size // 512):
                        nc.tensor.matmul(
                            ps[:, n * 512 : (n + 1) * 512],
                            ones,
                            sq[:, n * 512 : (n + 1) * 512],
                            start=(kt == 0),
                            stop=(kt == KT - 1),
                        )
                # inv-norm, replicated across partitions by the ones-matmul:
                # inv = exp(-0.5 * ln(ss)) (one ACT table set, full width)
                lnb = small_pool.tile([128, size], F32, tag="lnb", name="lnb")
                nc.scalar.activation(out=lnb, in_=ps[:, 0:size], func=AF.Ln)
                invb = invb_pool.tile([128, size], BF16, tag="invb", name="invb")
                nc.scalar.activation(out=invb, in_=lnb, func=AF.Exp, scale=-0.5)
                if pending is not None:
                    emit_scales(pending)
                pending = (cs, ztst, invb)
            emit_scales(pending)

            # ---- GEMM + fused exp/accumulate --------------------------------
            # logits chunk = znT_own(m).T @ znT_all(chunk); exp(2x) with
            # free-dim accumulation, written back in-place to PSUM.
            for nb in range(NCH):
                for m in range(MT):
                    ps = psum_pool.tile([128, CHUNK], F32)
                    for kt in range(KT):
                        lhsT = znT[kt][:, m * 128 : (m + 1) * 128]
                        for n in range(CHUNK // 512):
                            col = nb * CHUNK + n * 512
                            nc.tensor.matmul(
                                ps[:, n * 512 : (n + 1) * 512],
                                lhsT,
                                znT[kt][:, col : col + 512],
                                start=(kt == 0),
                                stop=(kt == KT - 1),
                            )
                    if nb < 2:
                        # stash exp(2*logit) of the target-diagonal window in
                        # SBUF before the in-place exp below overwrites PSUM;
                        # the diagonal is extracted after the scale stream.
                        nc.scalar.activation(
                            out=escr[:, nb, m, :],
                            in_=ps[:, m * 128 : (m + 1) * 128],
                            func=AF.Exp, scale=TEMP_INV,
                        )
                    nc.scalar.activation(
                        out=ps, in_=ps, func=AF.Exp, scale=TEMP_INV,
                        accum_out=acc[:, m, nb : nb + 1],
                    )
                if nb == 2:
                    # target-diagonal extraction (inputs ready after nb=1;
                    # emitted here so it overlaps the remaining GEMM instead
                    # of running in the kernel tail)
                    for w in range(2):
                        dtarget = d0 if w == 0 else d1
                        for m in range(MT):
                            dsc = sq_pool.tile([128, 128], F32, tag="dsc")
                            nc.vector.tensor_mul(dsc, escr[:, w, m, :], ident)
                            nc.vector.tensor_reduce(
                                out=dtarget[:, m : m + 1], in_=dsc,
                                axis=mybir.AxisListType.X, op=ALU.add,
                            )

            # d0/d1 hold exp(2*cos); recover the logit via ln
            nc.scalar.activation(out=d0, in_=d0, func=AF.Ln)
            nc.scalar.activation(out=d1, in_=d1, func=AF.Ln)
            t2a = persist.tile([128, MT], F32, tag="t2a")
            nc.vector.tensor_scalar_mul(t2a, d0, tselt[:, 0:1])
            t2b = persist.tile([128, MT], F32, tag="t2b")
            nc.vector.tensor_scalar_mul(t2b, d1, tselt[:, 1:2])
            t2 = persist.tile([128, MT], F32, tag="t2")
            nc.vector.tensor_add(t2, t2a, t2b)

            # ---- finalize: lse = ln(sum exp), partials = lse - t ---------
            ssum = persist.tile([128, MT], F32, tag="ssum")
            nc.vector.tensor_reduce(
                out=ssum, in_=acc, axis=mybir.AxisListType.X, op=ALU.add
            )
            lse = persist.tile([128, MT], F32, tag="lse")
            nc.scalar.activation(out=lse, in_=ssum, func=AF.Ln)
            diff = persist.tile([128, MT], F32, tag="diff")
            nc.vector.tensor_sub(diff, lse, t2)
            nc.sync.dma_start(out=out.ap(), in_=diff)

    _split_waits(nc)
    return nc


_nc_cache = None


def _get_nc():
    global _nc_cache
    if _nc_cache is None:
        _nc_cache = _build_nc()
    return _nc_cache


# ---------------------------------------------------------------------------
# Host wrapper: shard, run SPMD on cores 0-7, reduce
# ---------------------------------------------------------------------------
def kernel(features_1, features_2, _trace=False):
    f1 = np.ascontiguousarray(np.asarray(features_1, dtype=np.float32))
    f2 = np.ascontiguousarray(np.asarray(features_2, dtype=np.float32))
    assert f1.shape == (B, D) and f2.shape == (B, D)
    z = np.concatenate([f1, f2], axis=0)  # [N2, D]

    in_maps = []
    allrows = np.arange(N2)
    for c in range(NCORES):
        own_lo = c * ROWS
        lab_lo = (c % (B // ROWS)) * ROWS
        own_idx = allrows[own_lo : own_lo + ROWS]
        if lab_lo == own_lo:
            rest = np.concatenate([allrows[:own_lo], allrows[own_lo + ROWS :]])
            R = np.concatenate([own_idx, rest])
            sel = (1.0, 0.0)
        else:
            lab_idx = allrows[lab_lo : lab_lo + ROWS]
            keep = np.ones(N2, dtype=bool)
            keep[own_idx] = False
            keep[lab_idx] = False
            R = np.concatenate([own_idx, lab_idx, allrows[keep]])
            sel = (0.0, 1.0)
        in_maps.append(
            {
                "zt": np.ascontiguousarray(z[R].T),
                "tsel": np.tile(np.array(sel, np.float32), (128, 1)),
            }
        )

    nc = _get_nc()
    res = run_bass_kernel_spmd(
        nc, in_maps, core_ids=list(range(NCORES)), trace=_trace
    )
    total = np.float64(0.0)
    for c in range(NCORES):
        total += res.results[c]["out"].astype(np.float64).sum()
    loss = np.float32(total / N2)
    if _trace:
        return loss, res
    return loss


# revision 29
# speedup vs baseline: 1.0422x; 1.0143x over previous
"""Trainium2 Bass kernel for nn_ContrastiveLoss (SimCLR-style NT-Xent loss).

Math: z = concat(f1, f2) [2B, D]; zn = z / ||z||_row;
logits = zn @ zn.T / T; labels[i] = i mod B;
loss = mean_i(logsumexp(logits[i, :]) - logits[i, label_i]).

Distribution: data-parallel over rows of z across 8 NeuronCores. Each
core computes its 1024-row block of logits against all 8192 columns
(bf16 GEMM on the PE), with the softmax statistics fused on the fly:
exp(2*cos) with per-instruction free-dim accumulation on the Scalar
engine, so the full 8192x8192 logits matrix is never materialized.
The target logit is computed separately as an elementwise row-dot
(t_r = 2 * zn_r . zn_label(r)), so no gather is needed. Row norms are
computed on-device from the transposed operand via Square (ACT) +
ones-matmul partition reduction (PE). The host only does layout
(concat/slice/permute/transpose), sharding, and the final 8-way sum.
"""

import numpy as np

import concourse.bass as bass
import concourse.mybir as mybir
import concourse.tile as tile
from concourse.bass_utils import run_bass_kernel_spmd
from concourse.masks import make_identity
from concourse.vector_clock import ScopedClock

F32 = mybir.dt.float32
BF16 = mybir.dt.bfloat16
AF = mybir.ActivationFunctionType
ALU = mybir.AluOpType

B = 4096
D = 512
N2 = 2 * B          # 8192 rows of z
NCORES = 8
ROWS = N2 // NCORES  # 1024 rows per core
MT = ROWS // 128     # 8 m-tiles per core
KT = D // 128        # 4 k-tiles
CHUNK = 1024         # GEMM column chunk (2 PSUM banks)
NCH = N2 // CHUNK    # 8 GEMM column chunks
SCHUNK = 1024        # normalize/scale column chunk
NSC = N2 // SCHUNK   # 8 scale chunks
TEMP_INV = 2.0       # 1 / temperature


# ---------------------------------------------------------------------------
# Patches for this toolchain build:
# 1) walrus CoreV2/V3 codegen only accepts ONE sync wait per instruction;
#    Tile attaches several (tail drain, multi-dep DMAs). Split extras onto
#    standalone EventSemaphore instructions placed immediately before the
#    overloaded instruction (same engine, same basic block) — blocking at
#    engine-issue time is strictly more conservative and deadlock-free
#    because Tile's per-engine streams preserve global dependency order.
# ---------------------------------------------------------------------------
_MAX_WAITS = 1
_patched = False


def _patched_drain_and_barrier(self, tick_clock, wait_clock):
    nc = self.nc
    drain_inst = nc.sync.drain()
    wait_clock.add_sem_waits(
        drain_inst.ins, ScopedClock({None: tick_clock.global_clock})
    )
    si = drain_inst.ins.sync_info
    if si is not None and si.on_wait and len(si.on_wait) > _MAX_WAITS:
        waits = list(si.on_wait)
        si.on_wait = waits[:_MAX_WAITS]
        for i in range(_MAX_WAITS, len(waits), _MAX_WAITS):
            extra = nc.sync.drain()
            extra.ins.sync_info = mybir.SyncInfo(
                on_wait=waits[i : i + _MAX_WAITS], on_update=[]
            )
    nc.all_engine_barrier()
    assert self.sems is not None
    popped = nc._tile_sem_poison_stack.pop()
    assert popped is self._sem_poison
    nc.clear_and_free_semaphores(list(self.sems.allocated().values()))
    nc.all_engine_barrier()


def _apply_patches():
    global _patched
    if _patched:
        return
    tile.TileContext._drain_and_barrier = _patched_drain_and_barrier
    _patched = True


def _split_waits(nc):
    n = 0
    for fn in nc.m.functions:
        for bb in fn.blocks:
            insts = bb.instructions
            if not any(
                i.sync_info
                and i.sync_info.on_wait
                and len(i.sync_info.on_wait) > _MAX_WAITS
                for i in insts
            ):
                continue
            out = []
            for inst in insts:
                si = inst.sync_info
                if si and si.on_wait and len(si.on_wait) > _MAX_WAITS:
                    waits = list(si.on_wait)
                    for w in waits[:-_MAX_WAITS]:
                        n += 1
                        ev = mybir.InstEventSemaphore(
                            name=f"WSPLIT-{n}", ins=[], outs=[]
                        )
                        ev.engine = inst.engine
                        ev.sync_info = mybir.SyncInfo(on_wait=[w], on_update=[])
                        out.append(ev)
                    si.on_wait = waits[-_MAX_WAITS:]
                out.append(inst)
            bb.instructions = out
    return n


# ---------------------------------------------------------------------------
# Device kernel (identical program on all 8 cores; per-core data differs)
# ---------------------------------------------------------------------------
def _build_nc():
    _apply_patches()
    nc = bass.Bass()

    # zt:   [D, N2] f32 — z rows transposed; per-core column order:
    #       own rows first, then (cores 4-7) label rows, then the rest
    # tsel: [128, 2] f32 — one-hot: target diagonal at col 0 or col 1024
    zt = nc.declare_dram_parameter("zt", [D, N2], F32, isOutput=False)
    tsel = nc.declare_dram_parameter("tsel", [128, 2], F32, isOutput=False)
    out = nc.declare_dram_parameter("out", [128, MT], F32, isOutput=True)


    with tile.TileContext(nc) as tc:
        with (
            tc.tile_pool(name="persist", bufs=1) as persist,
            tc.tile_pool(name="ztst", bufs=8) as ztst_pool,
            tc.tile_pool(name="sq", bufs=4) as sq_pool,
            tc.tile_pool(name="invb", bufs=3) as invb_pool,
            tc.tile_pool(name="small", bufs=3) as small_pool,
            tc.tile_pool(name="psum", bufs=4, space="PSUM") as psum_pool,
        ):
            # persistent tensors
            znT = [
                persist.tile([128, N2], BF16, tag=f"znT{k}", name=f"znT{k}") for k in range(KT)
            ]
            ones = persist.tile([128, 128], BF16, tag="ones")
            nc.vector.memset(ones, 1.0)
            acc = persist.tile([128, MT, NCH], F32, tag="acc")
            d0 = persist.tile([128, MT], F32, tag="d0")
            d1 = persist.tile([128, MT], F32, tag="d1")
            ident = persist.tile([128, 128], BF16, tag="ident")
            make_identity(nc, ident)
            tselt = persist.tile([128, 2], F32, tag="tselt")
            nc.sync.dma_start(out=tselt, in_=tsel.ap())
            escr = persist.tile([128, 2, MT, 128], BF16, tag="escr")

            # ---- per column-chunk: cast-DMA zt to bf16, norms^2 via
            #      square (DVE) + ones-matmul (PE), inv = exp(-0.5*ln(ss))
            #      on ACT (replicated across partitions), scale to znT --
            # Software-pipelined by EMISSION order: chunk cc's scales are
            # emitted after chunk cc+1's squares, so in the scheduler's
            # priority order a later chunk's squares never head-of-line
            # block an earlier chunk's scales on the DVE.
            def emit_scales(pend):
                pcs, pztst, pinvb = pend
                for kt in range(KT):
                    nc.vector.tensor_mul(znT[kt][:, pcs], pztst[kt], pinvb)

            sizes = [SCHUNK] * NSC
            pending = None
            off = 0
            for cc, size in enumerate(sizes):
                cs = slice(off, off + size)
                off += size
                ps = psum_pool.tile([128, CHUNK], F32, name="ps")
                ztst = {}
                for kt in range(KT):
                    st = ztst_pool.tile([128, size], BF16, tag=f"zt{kt}", name=f"zt{kt}")
                    nc.gpsimd.dma_start(
                        out=st, in_=zt.ap()[kt * 128 : (kt + 1) * 128, cs]
                    )
                    ztst[kt] = st
                    sq = sq_pool.tile([128, size], BF16, tag="sq", name="sq")
                    nc.vector.tensor_mul(sq, st, st)
                    for n in range(size // 512):
                        nc.tensor.matmul(
                            ps[:, n * 512 : (n + 1) * 512],
                            ones,
                            sq[:, n * 512 : (n + 1) * 512],
                            start=(kt == 0),
                            stop=(kt == KT - 1),
                        )
                # inv-norm, replicated across partitions by the ones-matmul:
                # inv = exp(-0.5 * ln(ss)) (one ACT table set, full width)
                lnb = small_pool.tile([128, size], F32, tag="lnb", name="lnb")
                nc.scalar.activation(out=lnb, in_=ps[:, 0:size], func=AF.Ln)
                invb = invb_pool.tile([128, size], BF16, tag="invb", name="invb")
                nc.scalar.activation(out=invb, in_=lnb, func=AF.Exp, scale=-0.5)
                if pending is not None:
                    emit_scales(pending)
                pending = (cs, ztst, invb)
            emit_scales(pending)

            # ---- GEMM + fused exp/accumulate --------------------------------
            # logits chunk = znT_own(m).T @ znT_all(chunk); exp(2x) with
            # free-dim accumulation, written back in-place to PSUM.
            for nb in range(NCH):
                for m in range(MT):
                    ps = psum_pool.tile([128, CHUNK], F32)
                    for kt in range(KT):
                        lhsT = znT[kt][:, m * 128 : (m + 1) * 128]
                        for n in range(CHUNK // 512):
                            col = nb * CHUNK + n * 512
                            nc.tensor.matmul(
                                ps[:, n * 512 : (n + 1) * 512],
                                lhsT,
                                znT[kt][:, col : col + 512],
                                start=(kt == 0),
                                stop=(kt == KT - 1),
                            )
                    if nb < 2:
                        # stash exp(2*logit) of the target-diagonal window in
                        # SBUF before the in-place exp below overwrites PSUM;
                        # the diagonal is extracted after the scale stream.
                        nc.scalar.activation(
                            out=escr[:, nb, m, :],
                            in_=ps[:, m * 128 : (m + 1) * 128],
                            func=AF.Exp, scale=TEMP_INV,
                        )
                    nc.scalar.activation(
                        out=ps, in_=ps, func=AF.Exp, scale=TEMP_INV,
                        accum_out=acc[:, m, nb : nb + 1],
                    )
                if nb == 2:
                    # target-diagonal extraction (inputs ready after nb=1;
                    # emitted here so it overlaps the remaining GEMM instead
                    # of running in the kernel tail)
                    for w in range(2):
                        dtarget = d0 if w == 0 else d1
                        for m in range(MT):
                            dsc = sq_pool.tile([128, 128], F32, tag="dsc")
                            nc.vector.tensor_mul(dsc, escr[:, w, m, :], ident)
                            nc.vector.tensor_reduce(
                                out=dtarget[:, m : m + 1], in_=dsc,
                                axis=mybir.AxisListType.X, op=ALU.add,
                            )

            # d0/d1 hold exp(2*cos); recover the logit via ln
            nc.scalar.activation(out=d0, in_=d0, func=AF.Ln)
            nc.scalar.activation(out=d1, in_=d1, func=AF.Ln)
            t2a = persist.tile([128, MT], F32, tag="t2a")
            nc.vector.tensor_scalar_mul(t2a, d0, tselt[:, 0:1])
            t2b = persist.tile([128, MT], F32, tag="t2b")
            nc.vector.tensor_scalar_mul(t2b, d1, tselt[:, 1:2])
            t2 = persist.tile([128, MT], F32, tag="t2")
            nc.vector.tensor_add(t2, t2a, t2b)

            # ---- finalize: lse = ln(sum exp), partials = lse - t ---------
            ssum = persist.tile([128, MT], F32, tag="ssum")
            nc.vector.tensor_reduce(
                out=ssum, in_=acc, axis=mybir.AxisListType.X, op=ALU.add
            )
            lse = persist.tile([128, MT], F32, tag="lse")
            nc.scalar.activation(out=lse, in_=ssum, func=AF.Ln)
            diff = persist.tile([128, MT], F32, tag="diff")
            nc.vector.tensor_sub(diff, lse, t2)
            nc.sync.dma_start(out=out.ap(), in_=diff)

    _split_waits(nc)
    return nc


_nc_cache = None


def _get_nc():
    global _nc_cache
    if _nc_cache is None:
        _nc_cache = _build_nc()
    return _nc_cache


# ---------------------------------------------------------------------------
# Host wrapper: shard, run SPMD on cores 0-7, reduce
# ---------------------------------------------------------------------------
def kernel(features_1, features_2, _trace=False):
    f1 = np.ascontiguousarray(np.asarray(features_1, dtype=np.float32))
    f2 = np.ascontiguousarray(np.asarray(features_2, dtype=np.float32))
    assert f1.shape == (B, D) and f2.shape == (B, D)
    z = np.concatenate([f1, f2], axis=0)  # [N2, D]

    in_maps = []
    allrows = np.arange(N2)
    for c in range(NCORES):
        own_lo = c * ROWS
        lab_lo = (c % (B // ROWS)) * ROWS
        own_idx = allrows[own_lo : own_lo + ROWS]
        if lab_lo == own_lo:
            rest = np.concatenate([allrows[:own_lo], allrows[own_lo + ROWS :]])
            R = np.concatenate([own_idx, rest])
            sel = (1.0, 0.0)
        else:
            lab_idx = allrows[lab_lo : lab_lo + ROWS]
            keep = np.ones(N2, dtype=bool)
            keep[own_idx] = False
            keep[lab_idx] = False
            R = np.concatenate([own_idx, lab_idx, allrows[keep]])
            sel = (0.0, 1.0)
        in_maps.append(
            {
                "zt": np.ascontiguousarray(z[R].T),
                "tsel": np.tile(np.array(sel, np.float32), (128, 1)),
            }
        )

    nc = _get_nc()
    res = run_bass_kernel_spmd(
        nc, in_maps, core_ids=list(range(NCORES)), trace=_trace
    )
    total = np.float64(0.0)
    for c in range(NCORES):
        total += res.results[c]["out"].astype(np.float64).sum()
    loss = np.float32(total / N2)
    if _trace:
        return loss, res
    return loss
